# revision 1
# baseline (speedup 1.0000x reference)
# Causal self-attention (B=4, T=2048, C=1024, H=16, D=64) on 8 TRN2 NeuronCores.
#
# Sharding: core c = (batch b = c//2, head-half g = c%2) -> 8 heads of one batch.
# Each core computes the qkv projection for its head group, causal attention,
# and a rank-512 partial of the output projection. Host sums the two partials
# per batch and adds the constant vector W_proj @ b_v + b_proj (the k-bias is
# dropped: softmax is invariant to it; the v-bias commutes through the convex
# combination).
#
# On-core scheme (v2, bf16 matmuls, scores transposed):
#   qT/kT tiles hold two heads (partitions 0-63 / 64-127); S^T comes from K=64
#   matmul pairs packed on the PE array via row groups, h=0/h=1 written into
#   one 2-bank PSUM tile [128, 2, 512] so a single exp() covers both heads
#   (halves ACT instruction overhead). exp() needs no max-subtraction
#   (|S| <~ 2). The causal mask reduces to one shared [128, 2, 128] additive
#   tile applied only to the 128-wide diagonal boundary of each s-tile.
#   Row sums come free from an appended ones-column on V (M=65 AV matmuls);
#   normalization = DVE reciprocal + gpsimd partition_broadcast + DVE mult.
#   QKV for block n+1 is emitted interleaved into attention t-block j=n so the
#   PE-heavy projection work fills ACT-bound stalls of the exp pipeline, and
#   the output projection DMAs straight from PSUM (no SBUF staging copy).
import numpy as np
import ml_dtypes

B, T, C, H, D = 4, 2048, 1024, 16, 64
NEG = -30000.0
K5 = 5  # kt ring depth (blocks) per pair
V20 = 20  # vt ring depth (s-tiles)

_NC = {}


_TABLES_PATCHED = False


def _pin_act_table():
    # The act-table-load pass binds each activation to the first set
    # containing its function, so mixing Exp (exp_and_others) with Ln
    # (natural_log_exp_and_others) reloads tables at every normalize.
    # Strip Exp/Ln from every other set (indices/order preserved, so the
    # act_func_set_id -> act_info.json mapping stays valid): all Exp+Ln
    # activations then share natural_log_exp_and_others -> one load total.
    global _TABLES_PATCHED
    if _TABLES_PATCHED:
        return
    import concourse.hw_specs as hw_specs
    import concourse.bacc as bacc
    import concourse.bass_interp as bass_interp
    from concourse import mybir

    AF = mybir.ActivationFunctionType
    orig = hw_specs.get_activation_tables

    def patched(arch):
        tabs = orig(arch)
        return {
            name: (fs if name == "natural_log_exp_and_others" else fs - {AF.Exp, AF.Ln})
            for name, fs in tabs.items()
        }

    hw_specs.get_activation_tables = patched
    bacc.get_activation_tables = patched
    bass_interp.get_activation_tables = patched
    _TABLES_PATCHED = True


def _build(reps=1):
    import concourse.bacc as bacc
    import concourse.tile as tile
    from concourse import mybir

    _pin_act_table()

    BF16 = mybir.dt.bfloat16
    F32 = mybir.dt.float32
    AF = mybir.ActivationFunctionType
    ALU = mybir.AluOpType

    nc = bacc.Bacc("TRN2", target_bir_lowering=False, debug=False, num_devices=8)
    xT = nc.dram_tensor("xT", [C, T], BF16, kind="ExternalInput")
    wqT = nc.dram_tensor("wqT", [C, 512], BF16, kind="ExternalInput")
    wkT = nc.dram_tensor("wkT", [C, 512], BF16, kind="ExternalInput")
    wvT = nc.dram_tensor("wvT", [C, 512], BF16, kind="ExternalInput")
    wpT = nc.dram_tensor("wpT", [512, C], BF16, kind="ExternalInput")
    bq2 = nc.dram_tensor("bq2", [4, 128], F32, kind="ExternalInput")
    mask2 = nc.dram_tensor("mask2", [128, 2, 128], F32, kind="ExternalInput")
    out = nc.dram_tensor("out", [T, C], F32, kind="ExternalOutput")

    NJ = T // 512  # t blocks per rep
    NB = NJ * reps  # global block count

    with tile.TileContext(nc) as tc:
        with (
            tc.tile_pool(name="const", bufs=1) as const,
            tc.tile_pool(name="xq_p", bufs=3) as xq_p,
            tc.tile_pool(name="qt_p", bufs=8) as qt_p,
            tc.tile_pool(name="pt_p", bufs=4) as pt_p,
            tc.tile_pool(name="ot_p", bufs=8) as ot_p,
            tc.tile_pool(name="sm_p", bufs=2) as sm_p,
            tc.tile_pool(name="ob_p", bufs=2) as ob_p,
            tc.tile_pool(name="on_p", bufs=4) as on_p,
            tc.tile_pool(name="ps_a", bufs=2, space="PSUM") as ps_a,
            tc.tile_pool(name="ps_st", bufs=2, space="PSUM") as ps_st,
            tc.tile_pool(name="ps_o", bufs=1, space="PSUM") as ps_o,
        ):
            # resident weights / constants
            wq_sb = const.tile([128, 8, 512], BF16)
            wk_sb = const.tile([128, 8, 512], BF16)
            wv_sb = const.tile([128, 8, 512], BF16)
            wp_sb = const.tile([128, 4, C], BF16)
            mk_sb = const.tile([128, 2, 128], F32)
            bq_sb = const.tile([128, 4], F32)
            ones_sb = const.tile([128, 8], BF16)
            nc.vector.memset(ones_sb[:], 1.0)

            kt = {
                (p, b): const.tile([128, 512], BF16, name=f"kt{p}_{b}", tag=f"kt{p}_{b}")
                for p in range(4)
                for b in range(K5)
            }
            vt = [
                const.tile([128, 8, 65], BF16, name=f"vt{i}", tag=f"vt{i}")
                for i in range(V20)
            ]

            xTr = xT.rearrange("(kt p) t -> p kt t", p=128)

            # ---- DMA prologue, ordered by first use ----
            def xq_dma(nb):
                xq = []
                for half in range(2):
                    xh = xq_p.tile(
                        [128, 4, 512], BF16, name=f"xq_{nb}_{half}", tag=f"xq{half}"
                    )
                    n = nb % NJ
                    nc.sync.dma_start(
                        xh[:],
                        xTr[:, 4 * half : 4 * half + 4, 512 * n : 512 * (n + 1)],
                    )
                    xq.append(xh)
                return xq

            # startup: stage DMAs in first-use order so the first QKV octet
            # (which only needs x half 0 + wq cols 0:128) starts ASAP
            wqr = wqT.rearrange("(kt p) m -> p kt m", p=128)
            wkr = wkT.rearrange("(kt p) m -> p kt m", p=128)
            xq_cur = [None, None]

            def xq_dma_half(nb, half):
                xh = xq_p.tile(
                    [128, 4, 512], BF16, name=f"xq_{nb}_{half}", tag=f"xq{half}"
                )
                n = nb % NJ
                nc.sync.dma_start(
                    xh[:], xTr[:, 4 * half : 4 * half + 4, 512 * n : 512 * (n + 1)]
                )
                return xh

            xq_cur[0] = xq_dma_half(0, 0)
            nc.sync.dma_start(wq_sb[:, :, 0:256], wqr[:, :, 0:256])
            xq_cur[1] = xq_dma_half(0, 1)
            nc.sync.dma_start(wk_sb[:, :, 0:256], wkr[:, :, 0:256])
            nc.sync.dma_start(bq_sb[:], bq2.rearrange("m p -> p m"))
            nc.sync.dma_start(wq_sb[:, :, 256:], wqr[:, :, 256:])
            nc.sync.dma_start(wk_sb[:, :, 256:], wkr[:, :, 256:])
            nc.sync.dma_start(wv_sb[:], wvT.rearrange("(kt p) m -> p kt m", p=128))
            nc.sync.dma_start(mk_sb[:], mask2[:])
            nc.sync.dma_start(wp_sb[:], wpT.rearrange("(pr p) co -> p pr co", p=128))

            qt = {}

            # ---- QKV octet emitters for one global block nb ----
            def emit_q(nb, mt, xq):
                psq = ps_a.tile([128, 512], F32, tag="ps_a")
                for k in range(8):
                    nc.tensor.matmul(
                        psq[:],
                        wq_sb[:, k, 128 * mt : 128 * (mt + 1)],
                        xq[k // 4][:, k % 4, :],
                        start=(k == 0),
                        stop=(k == 7),
                    )
                q_tile = qt_p.tile([128, 512], BF16, name=f"qt_{nb}_{mt}", tag="qt")
                qt[(mt, nb)] = q_tile
                nc.vector.tensor_scalar_add(q_tile[:], psq[:], bq_sb[:, mt : mt + 1])

            def emit_k(nb, mt, xq):
                psk = ps_a.tile([128, 512], F32, tag="ps_a")
                for k in range(8):
                    nc.tensor.matmul(
                        psk[:],
                        wk_sb[:, k, 128 * mt : 128 * (mt + 1)],
                        xq[k // 4][:, k % 4, :],
                        start=(k == 0),
                        stop=(k == 7),
                    )
                nc.vector.tensor_copy(kt[(mt, nb % K5)][:], psk[:])

            def emit_v(nb, tt, xq):
                psv = ps_a.tile([128, 512], F32, tag="ps_a")
                for k in range(8):
                    nc.tensor.matmul(
                        psv[:],
                        xq[k // 4][:, k % 4, 128 * tt : 128 * (tt + 1)],
                        wv_sb[:, k, :],
                        start=(k == 0),
                        stop=(k == 7),
                    )
                si = 4 * nb + tt
                nc.vector.tensor_copy(
                    vt[si % V20][:, :, 0:64],
                    psv.rearrange("p (h d) -> p h d", d=64),
                )
                nc.vector.tensor_copy(vt[si % V20][:, :, 64], ones_sb[:])

            def qkv_octets(nb, xq):
                for mt in range(4):
                    yield lambda mt=mt: emit_q(nb, mt, xq)
                    yield lambda mt=mt: emit_k(nb, mt, xq)
                for tt in range(4):
                    yield lambda tt=tt: emit_v(nb, tt, xq)

            def proj_group(nb, ot2, tt, half):
                j = nb % NJ
                pp = ps_a.tile([128, 512], F32, tag="ps_a")
                for pair in range(4):
                    nc.tensor.matmul(
                        pp[:],
                        ot2[pair][:, 128 * tt : 128 * (tt + 1)],
                        wp_sb[:, pair, 512 * half : 512 * (half + 1)],
                        start=(pair == 0),
                        stop=(pair == 3),
                    )
                ob = ob_p.tile([128, 512], F32, tag="ob")
                nc.vector.tensor_copy(ob[:], pp[:])
                nc.sync.dma_start(
                    out[
                        512 * j + 128 * tt : 512 * j + 128 * (tt + 1),
                        512 * half : 512 * (half + 1),
                    ],
                    ob[:],
                )

            def proj_groups(nb, ot2):
                for tt in range(4):
                    for half in range(2):
                        yield lambda tt=tt, half=half: proj_group(nb, ot2, tt, half)

            # QKV for block 0 is emitted up-front (PE is busy regardless;
            # attention needs its outputs immediately)
            for em in qkv_octets(0, xq_cur):
                em()
            xq_tiles = {}
            if NB > 1:
                xq_tiles[1] = xq_dma(1)

            # ---- attention per global block. PE filler (QKV octets of the
            # next block, delayed output-projection groups) is placed so
            # every block stays PE-bound: late (big, ACT-heavy) attention
            # blocks get the most filler. For a rep's LAST block, its own
            # QKV is split — q(pair0) ahead of time, k/v inside pair 0's
            # early si window, q(pair p) right before pair p starts. ----
            LAST = NJ - 1
            proj_delayed = []  # (release_at_nb, emitter)
            defer_kv = {}  # nb -> list of k/v octet emitters
            bound_q = {}  # nb -> {pair: q emitter}
            spread = []
            for nb in range(NB):
                rep, j = divmod(nb, NJ)
                # prefetch x two blocks ahead; queue next block's QKV
                if nb + 2 < NB:
                    xq_tiles[nb + 2] = xq_dma(nb + 2)
                if nb + 1 < NB:
                    xq_n = xq_tiles.pop(nb + 1)
                    nxt = nb + 1
                    if nxt % NJ == LAST:
                        spread.append(lambda n=nxt, x=xq_n: emit_q(n, 0, x))
                        defer_kv[nxt] = [
                            (lambda mt=mt, n=nxt, x=xq_n: emit_k(n, mt, x))
                            for mt in range(4)
                        ] + [
                            (lambda tt=tt, n=nxt, x=xq_n: emit_v(n, tt, x))
                            for tt in range(4)
                        ]
                        bound_q[nxt] = {
                            p: (lambda p=p, n=nxt, x=xq_n: emit_q(n, p, x))
                            for p in (1, 2, 3)
                        }
                    else:
                        spread.extend(qkv_octets(nxt, xq_n))
                # release delayed projections scheduled for this block
                for rel, em in [pd for pd in proj_delayed if pd[0] <= nb]:
                    spread.append(em)
                proj_delayed = [pd for pd in proj_delayed if pd[0] > nb]

                ns_live = 4 * (j + 1)
                n_slots = 4 * ns_live
                n_units = len(spread)
                emitted = 0
                slot = 0
                kv_units = defer_kv.pop(nb, [])
                kv_emitted = 0
                bq_units = bound_q.pop(nb, {})

                ot2 = []
                onums = []
                lns4 = []
                for pair in range(4):
                    if pair in bq_units:
                        bq_units.pop(pair)()
                    oaug = ps_o.tile(
                        [65, 2, 512], F32, name=f"oaug_{nb}_{pair}", tag="ps_o"
                    )
                    for si in range(ns_live):
                        gsi = 16 * rep + si
                        r = si - 4 * j
                        off = 128 * r if r > 0 else 0
                        st2 = ps_st.tile([128, 2, 512], F32, tag="st")
                        for h in range(2):
                            nc.tensor.matmul(
                                st2[:, h, off:],
                                kt[(pair, (si // 4 + rep * NJ) % K5)][
                                    64 * h : 64 * h + 64,
                                    128 * (si % 4) : 128 * (si % 4) + 128,
                                ],
                                qt[(pair, nb)][64 * h : 64 * h + 64, off:],
                                start=True,
                                stop=True,
                            )
                        if r >= 0:
                            nc.vector.tensor_tensor(
                                st2[:, :, off : off + 128],
                                st2[:, :, off : off + 128],
                                mk_sb[:],
                                ALU.add,
                            )
                        pt = pt_p.tile([128, 2, 512], BF16, tag="pt")
                        nc.scalar.activation(pt[:, :, off:], st2[:, :, off:], AF.Exp)
                        for h in range(2):
                            nc.tensor.matmul(
                                oaug[:, h, off:],
                                vt[gsi % V20][:, 2 * pair + h, :],
                                pt[:, h, off:],
                                start=(si == 0),
                                stop=(si == ns_live - 1),
                            )
                        # deferred k/v of THIS block: finish within pair 0's
                        # si < 4j window (needed from si = 4j on)
                        if kv_units and pair == 0 and j > 0:
                            tgt = ((si + 1) * 8) // (4 * j)
                            while kv_units and kv_emitted < tgt:
                                kv_units.pop(0)()
                                kv_emitted += 1
                        # drain spread filler evenly across the block's slots
                        slot += 1
                        while spread and emitted < (slot * n_units) // n_slots:
                            spread.pop(0)()
                            emitted += 1

                    o_tile = ot_p.tile(
                        [128, 512], BF16, name=f"ot_{nb}_{pair}", tag="ot"
                    )
                    ot2.append(o_tile)
                    if nb + 1 < NB:
                        # release the PSUM accumulator fast: ln(sums) on ACT
                        # + numerator copy-out; exp/broadcast/multiply are
                        # batched at block end, off the critical path (the
                        # projection that consumes o_tile is deferred anyway)
                        lns = sm_p.tile([1, 2, 512], F32, tag=f"lns{pair % 2}")
                        lns4.append(lns)
                        nc.scalar.activation(lns[:], oaug[64:65, :, :], AF.Ln)
                        onum = on_p.tile(
                            [64, 2, 512], BF16, name=f"onum_{nb}_{pair}", tag="onum"
                        )
                        onums.append(onum)
                        nc.vector.tensor_copy(onum[:], oaug[0:64, :, :])
                    else:
                        # last block: normalize inline to keep the kernel
                        # tail (… -> o -> proj -> out) short
                        lnv = sm_p.tile([1, 2, 512], F32, tag="lnv")
                        nc.scalar.activation(lnv[:], oaug[64:65, :, :], AF.Ln)
                        rec = sm_p.tile([1, 2, 512], F32, tag="rec")
                        nc.scalar.activation(rec[:], lnv[:], AF.Exp, scale=-1.0)
                        bc = sm_p.tile([64, 2, 512], F32, tag="bc")
                        nc.gpsimd.partition_broadcast(bc[:], rec[:])
                        for h in range(2):
                            nc.vector.tensor_tensor(
                                o_tile[64 * h : 64 * h + 64, :],
                                oaug[0:64, h, :],
                                bc[:, h, :],
                                ALU.mult,
                            )
                # drain leftovers; delay this block's projection so it lands
                # as filler in a later (bigger, ACT-heavier) block
                while kv_units:
                    kv_units.pop(0)()
                while spread and emitted < n_units:
                    spread.pop(0)()
                    emitted += 1
                if nb + 1 < NB:
                    # deferred normalize for all 4 pairs (SBUF-only
                    # multiplies run in the DVE 2x mode)
                    for pair in range(4):
                        rec = sm_p.tile([1, 2, 512], F32, tag=f"rec{pair % 2}")
                        nc.scalar.activation(rec[:], lns4[pair][:], AF.Exp, scale=-1.0)
                        bc = sm_p.tile([64, 2, 512], F32, tag=f"bc{pair % 2}")
                        nc.gpsimd.partition_broadcast(bc[:], rec[:])
                        for h in range(2):
                            nc.vector.tensor_tensor(
                                ot2[pair][64 * h : 64 * h + 64, :],
                                onums[pair][:, h, :],
                                bc[:, h, :],
                                ALU.mult,
                            )
                    rel = nb + 2 if j == 0 else nb + 1
                    proj_delayed.extend(
                        (rel, em) for em in proj_groups(nb, ot2)
                    )
                else:
                    for _, em in proj_delayed:
                        em()
                    for em in proj_groups(nb, ot2):
                        em()
    nc.compile()
    return nc


def _get_nc(reps=1):
    if reps not in _NC:
        _NC[reps] = _build(reps)
    return _NC[reps]


def _host_mask():
    i = np.arange(128)[:, None]
    c = np.arange(128)[None, :]
    m1 = np.where(c >= i, 0.0, NEG).astype(np.float32)
    return np.ascontiguousarray(np.broadcast_to(m1[:, None, :], (128, 2, 128)))


def _in_maps(x, W_attn, b_attn, W_proj):
    bf = ml_dtypes.bfloat16
    Wq, Wk, Wv = W_attn[0:C], W_attn[C : 2 * C], W_attn[2 * C : 3 * C]
    mask2 = _host_mask()
    g_in = []
    for g in range(2):
        sl = slice(512 * g, 512 * (g + 1))
        g_in.append(
            dict(
                wqT=np.ascontiguousarray(Wq[sl].T * 0.125).astype(bf),
                wkT=np.ascontiguousarray(Wk[sl].T).astype(bf),
                wvT=np.ascontiguousarray(Wv[sl].T).astype(bf),
                wpT=np.ascontiguousarray(W_proj[:, sl].T).astype(bf),
                bq2=(b_attn[sl] * 0.125).reshape(4, 128).astype(np.float32),
                mask2=mask2,
            )
        )
    xTs = [np.ascontiguousarray(x[b].T).astype(bf) for b in range(B)]
    return [dict(xT=xTs[c // 2], **g_in[c % 2]) for c in range(8)]


def kernel(x, W_attn, b_attn, W_proj, b_proj):
    from concourse.bass_utils import run_bass_kernel_spmd

    x = np.asarray(x, dtype=np.float32)
    W_attn = np.asarray(W_attn, dtype=np.float32)
    b_attn = np.asarray(b_attn, dtype=np.float32)
    W_proj = np.asarray(W_proj, dtype=np.float32)
    b_proj = np.asarray(b_proj, dtype=np.float32)

    nc = _get_nc()
    in_maps = _in_maps(x, W_attn, b_attn, W_proj)
    res = run_bass_kernel_spmd(nc, in_maps, core_ids=list(range(8)))

    cvec = (W_proj @ b_attn[2 * C : 3 * C] + b_proj).astype(np.float32)
    y = np.empty((B, T, C), np.float32)
    for b in range(B):
        y[b] = res.results[2 * b]["out"] + res.results[2 * b + 1]["out"] + cvec
    return y

